# revision 1
# baseline (speedup 1.0000x reference)
"""DiGCN inception-block GNN on 8 TRN2 NeuronCores.

Strategy: shard nodes (and their incoming edges) across 8 cores. Per layer:
  x_next = x@lnW + lnb + A1@(x@c1W) + c1b + A2@(x@c2W) + c2b
Since the 128x128 weights commute past the segment-sum, each core gathers raw
bf16 x rows for its edges (dma_gather, 4 SWDGE queues so all 8 Q7 cores
generate descriptors concurrently), scatter-sums them into per-128-node blocks
with a weighted-one-hot matmul accumulated in PSUM (G^T = sum M^T@O), then
applies the three weight matrices per block in a single PSUM accumulation
group. Node features are exchanged between layers with an AllGather.
"""

import sys

sys.path.insert(0, "/opt/trn_rl_repo")

import numpy as np
import ml_dtypes

import concourse.mybir as mybir
import concourse.tile as tile
from concourse import bacc
from concourse import bass_utils

# problem constants (hardcoded per the harness contract)
N = 50000
E = 500000
F = 128
L = 3
NC = 8
P = 128
SH = N // NC          # 6250 nodes per core
BLK = 49              # node blocks per core (49*128 = 6272)
SHP = BLK * P         # 6272 padded shard rows
NFULL = NC * SHP      # 50176 padded full rows
HALF = NFULL // 2     # 25088 (< 32768 so int16 indices work per half)
GB = 7                # blocks per metadata group
NG = BLK // GB        # 7 groups
CALL_T = 8            # tiles per dma_gather call (8*128 = 1024 idx max for
                      # single_packet=True)

BF16 = ml_dtypes.bfloat16


def _pad_row(node):
    return (node // SH) * SHP + (node % SH)


def _prep_edge_set(src, dst, w):
    """Partition one edge set by destination core/block, split by source half."""
    src = np.asarray(src).astype(np.int64)
    dst = np.asarray(dst).astype(np.int64)
    w = np.asarray(w).astype(np.float32)

    core = dst // SH
    blk = (dst % SH) // P
    dloc = (dst % SH) % P
    prow = _pad_row(src)
    half = (prow >= HALF).astype(np.int64)
    idx16 = (prow - half * HALF).astype(np.int64)

    key = (core * BLK + blk) * 2 + half
    order = np.argsort(key, kind="stable")
    key_s = key[order]
    counts = np.bincount(key, minlength=NC * BLK * 2).reshape(NC, BLK, 2)
    T = np.maximum(1, -(-counts // P)).max(axis=0)  # [BLK, 2]

    tile_off = np.zeros((BLK, 2), np.int64)
    tot = [0, 0]
    for h in (0, 1):
        for b in range(BLK):
            tile_off[b, h] = tot[h]
            tot[h] += T[b, h]
    nslots = [int(tot[0]) * P, int(tot[1]) * P]

    starts = np.concatenate([[0], np.cumsum(np.bincount(key_s, minlength=NC * BLK * 2))])
    per_core = []
    for c in range(NC):
        idx_sl = [np.zeros(nslots[h], np.int64) for h in (0, 1)]
        dst_sl = [np.zeros(nslots[h], np.float32) for h in (0, 1)]
        w_sl = [np.zeros(nslots[h], np.float32) for h in (0, 1)]
        for h in (0, 1):
            for b in range(BLK):
                k = (c * BLK + b) * 2 + h
                lo, hi = starts[k], starts[k + 1]
                n = hi - lo
                s0 = tile_off[b, h] * P
                sel = order[lo:hi]
                idx_sl[h][s0:s0 + n] = idx16[sel]
                dst_sl[h][s0:s0 + n] = dloc[sel]
                w_sl[h][s0:s0 + n] = w[sel]
        per_core.append({"idx": idx_sl, "dstf": dst_sl, "wf": w_sl})
    return T, tile_off, per_core


def _wrap_idx(idx_slots):
    n = idx_slots.shape[0]
    a = idx_slots.astype(np.int16).reshape(n // 16, 16).T
    return np.tile(a, (8, 1)).copy()


class _Plan:
    """Compile-time structure shared by all cores (derived from global data)."""

    def __init__(self, T1, off1, T2, off2):
        self.T = [T1, T2]          # [set][BLK, 2] tiles per (block, half)
        self.off = [off1, off2]    # tile offset within (set, half) stream
        self.tot = [[int(T[:, h].sum()) for h in (0, 1)] for T in self.T]
        # metadata groups: per (set, half, group) -> (tile_start, tile_count)
        self.groups = [[[None] * NG for _ in (0, 1)] for _ in (0, 1)]
        for s in (0, 1):
            for h in (0, 1):
                for g in range(NG):
                    b0, b1 = g * GB, (g + 1) * GB
                    t0 = int(self.off[s][b0, h])
                    t1 = int(self.off[s][b1 - 1, h] + self.T[s][b1 - 1, h])
                    self.groups[s][h][g] = (t0, t1 - t0)
        # gather calls: per (set, half) a list of (tile_start, tile_count)
        # covering the whole stream in chunks of <= CALL_T tiles, aligned to
        # group boundaries so buffers retire with their group.
        self.calls = [[[] for _ in (0, 1)] for _ in (0, 1)]
        self.call_of_tile = [[{} for _ in (0, 1)] for _ in (0, 1)]
        for s in (0, 1):
            for h in (0, 1):
                for g in range(NG):
                    t0, tcnt = self.groups[s][h][g]
                    c = t0
                    while c < t0 + tcnt:
                        n = min(CALL_T, t0 + tcnt - c)
                        ci = len(self.calls[s][h])
                        self.calls[s][h].append((c, n, g))
                        for t in range(c, c + n):
                            self.call_of_tile[s][h][t] = (ci, t - c)
                        c += n
        self.hbase = [[0, self.tot[s][0]] for s in (0, 1)]
        self.ncols = [self.tot[s][0] + self.tot[s][1] for s in (0, 1)]
        self.tg_max = max(
            self.groups[s][h][g][1] for s in (0, 1) for h in (0, 1) for g in range(NG)
        )


def _build_nc(plan: _Plan, n_layers=L, use_collective=True, n_groups=NG):
    nc = bacc.Bacc("TRN2", target_bir_lowering=False, debug=False,
                   enable_asserts=True, num_devices=NC, num_swdge_queues=4)
    dt = mybir.dt

    xb_t = nc.dram_tensor("xb", [NFULL, F], dt.bfloat16, kind="ExternalInput")
    xown_t = nc.dram_tensor("xown", [SHP, F], dt.bfloat16, kind="ExternalInput")
    wall_t = nc.dram_tensor("wall", [P, 9 * F], dt.bfloat16, kind="ExternalInput")
    bsum_t = nc.dram_tensor("bsum", [P, L], dt.float32, kind="ExternalInput")
    iota_t = nc.dram_tensor("iota", [P, P], dt.bfloat16, kind="ExternalInput")
    ident_t = nc.dram_tensor("ident", [P, P], dt.bfloat16, kind="ExternalInput")
    idx_ts = [[nc.dram_tensor(f"idx{s}{h}", [P, plan.tot[s][h] * 8], dt.int16,
                              kind="ExternalInput") for h in (0, 1)] for s in (0, 1)]
    # dst/w columns in bf16, one column per tile (h0 stream then h1 stream)
    dst_ts = [nc.dram_tensor(f"dst{s}", [P, plan.ncols[s]], dt.bfloat16,
                             kind="ExternalInput") for s in (0, 1)]
    w_ts = [nc.dram_tensor(f"w{s}", [P, plan.ncols[s]], dt.bfloat16,
                           kind="ExternalInput") for s in (0, 1)]
    out_t = nc.dram_tensor("outT", [P, SHP], dt.float32, kind="ExternalOutput")

    with tile.TileContext(nc) as tc:
        with tc.tile_pool(name="const", bufs=1) as constp, \
             tc.tile_pool(name="xt", bufs=2 * BLK) as xtp, \
             tc.tile_pool(name="msg", bufs=8) as msgp, \
             tc.tile_pool(name="ohp", bufs=8) as ohp, \
             tc.tile_pool(name="meta", bufs=2) as metap, \
             tc.tile_pool(name="stage", bufs=4) as stagep, \
             tc.tile_pool(name="gps", bufs=4, space="PSUM") as gpsp, \
             tc.tile_pool(name="ops", bufs=2, space="PSUM") as opsp, \
             tc.tile_pool(name="tps", bufs=2, space="PSUM") as tpsp, \
             tc.tile_pool(name="dram", bufs=2, space="DRAM") as dramp:

            iota_sb = constp.tile([P, P], dt.bfloat16)
            nc.sync.dma_start(out=iota_sb[:], in_=iota_t.ap())
            ident_sb = constp.tile([P, P], dt.bfloat16)
            nc.sync.dma_start(out=ident_sb[:], in_=ident_t.ap())
            wall_sb = constp.tile([P, 9 * F], dt.bfloat16)
            nc.sync.dma_start(out=wall_sb[:], in_=wall_t.ap())
            bsum_sb = constp.tile([P, L], dt.float32)
            nc.sync.dma_start(out=bsum_sb[:], in_=bsum_t.ap())

            def wsl(l, k):  # lhsT slice for layer l, kind k (0=ln,1=c1,2=c2)
                c0 = (l * 3 + k) * F
                return wall_sb[:, c0:c0 + F]

            xt_tiles = {}
            agin = [None, None]
            xfull = [None, None]
            for l in range(2):
                agin[l] = dramp.tile([SHP, F], dt.bfloat16, name=f"agin{l}")
                xfull[l] = dramp.tile([NFULL, F], dt.bfloat16,
                                      addr_space="Shared", name=f"xfull{l}")

            for l in range(n_layers):
                src_ap = xb_t.ap() if l == 0 else xfull[l - 1][:]
                src_half = [src_ap[0:HALF, :], src_ap[HALF:NFULL, :]]
                # sbuf tiles per (s, h): {call_index: (msg_tile, oh_tile)}
                call_tiles = [[{} for _ in (0, 1)] for _ in (0, 1)]
                meta = [[None, None], [None, None]]

                def emit_group_meta(g):
                    for s in (0, 1):
                        for h in (0, 1):
                            t0, tcnt = plan.groups[s][h][g]
                            c0 = plan.hbase[s][h] + t0
                            dc = metap.tile([P, plan.tg_max], dt.bfloat16,
                                            tag=f"dst{s}{h}",
                                            name=f"dst_{l}_{g}_{s}_{h}")
                            nc.sync.dma_start(out=dc[:, :tcnt],
                                              in_=dst_ts[s][:, c0:c0 + tcnt])
                            wc = metap.tile([P, plan.tg_max], dt.bfloat16,
                                            tag=f"wgt{s}{h}",
                                            name=f"w_{l}_{g}_{s}_{h}")
                            nc.sync.dma_start(out=wc[:, :tcnt],
                                              in_=w_ts[s][:, c0:c0 + tcnt])
                            meta[s][h] = (dc, wc, t0)

                def emit_calls_for_group(g):
                    # interleave the 4 streams' calls so the 4 SWDGE queues
                    # (one Q7 core pair each) work concurrently
                    percall = []
                    for s in (0, 1):
                        for h in (0, 1):
                            lst = [(ci, c) for ci, c in
                                   enumerate(plan.calls[s][h]) if c[2] == g]
                            percall.append((s, h, lst))
                    maxn = max(len(x[2]) for x in percall)
                    for i in range(maxn):
                        for s, h, lst in percall:
                            if i >= len(lst):
                                continue
                            ci, (t0, tcnt, _) = lst[i]
                            idx_sb = metap.tile(
                                [P, CALL_T * 8], dt.int16, tag=f"idx{s}{h}",
                                name=f"idx_{l}_{s}_{h}_{ci}", bufs=8)
                            nc.sync.dma_start(
                                out=idx_sb[:, :tcnt * 8],
                                in_=idx_ts[s][h][:, t0 * 8:(t0 + tcnt) * 8])
                            m = msgp.tile([P, CALL_T, F], dt.bfloat16,
                                          tag=f"msg{s}{h}",
                                          name=f"msg_{l}_{s}_{h}_{ci}")
                            nc.gpsimd.dma_gather(
                                out_ap=m[:, :tcnt, :],
                                in_ap=src_half[h],
                                idxs_ap=idx_sb[:, :tcnt * 8],
                                num_idxs=tcnt * P,
                                num_idxs_reg=tcnt * P,
                                elem_size=F,
                                single_packet=True,
                                queue_num=s * 2 + h,
                            )
                            # weighted one-hot for the call's tiles: 2 big TT
                            dc, wc, gt0 = meta[s][h]
                            mc0 = t0 - gt0
                            oh = ohp.tile([P, CALL_T, P], dt.bfloat16,
                                          tag=f"oh{s}{h}",
                                          name=f"oh_{l}_{s}_{h}_{ci}")
                            iota_b = iota_sb[:].unsqueeze(1).to_broadcast(
                                [P, tcnt, P])
                            nc.vector.tensor_tensor(
                                out=oh[:, :tcnt, :], in0=iota_b,
                                in1=dc[:, mc0:mc0 + tcnt].to_broadcast(
                                    [P, tcnt, P]),
                                op=mybir.AluOpType.is_equal)
                            nc.vector.tensor_tensor(
                                out=oh[:, :tcnt, :], in0=oh[:, :tcnt, :],
                                in1=wc[:, mc0:mc0 + tcnt].to_broadcast(
                                    [P, tcnt, P]),
                                op=mybir.AluOpType.mult)
                            call_tiles[s][h][ci] = (m, oh)

                for g in range(n_groups):
                    emit_group_meta(g)
                    emit_calls_for_group(g)

                    for b in range(g * GB, (g + 1) * GB):
                        gs = []
                        for s in (0, 1):
                            gp = gpsp.tile([P, P], dt.float32, tag="gp",
                                           name=f"gp_{l}_{b}_{s}")
                            ntiles = int(plan.T[s][b, 0] + plan.T[s][b, 1])
                            ti = 0
                            for h in (0, 1):
                                tb0 = int(plan.off[s][b, h])
                                for t in range(tb0, tb0 + int(plan.T[s][b, h])):
                                    ci, lt = plan.call_of_tile[s][h][t]
                                    m, oh = call_tiles[s][h][ci]
                                    nc.tensor.matmul(
                                        out=gp[:],
                                        lhsT=m[:, lt, :],
                                        rhs=oh[:, lt, :],
                                        start=(ti == 0),
                                        stop=(ti == ntiles - 1),
                                    )
                                    ti += 1
                            gsb = stagep.tile([P, P], dt.bfloat16, tag="gs",
                                              name=f"gs_{l}_{b}_{s}")
                            nc.scalar.copy(out=gsb[:], in_=gp[:])
                            gs.append(gsb)

                        if l == 0:
                            ld = stagep.tile([P, P], dt.bfloat16, tag="ld",
                                             name=f"ld_{b}")
                            nc.sync.dma_start(
                                out=ld[:], in_=xown_t.ap()[b * P:(b + 1) * P, :])
                            tp = tpsp.tile([P, P], dt.bfloat16, tag="tp",
                                           name=f"tp0_{b}")
                            nc.tensor.transpose(out=tp[:], in_=ld[:],
                                                identity=ident_sb[:])
                            xt_b = xtp.tile([P, P], dt.bfloat16, tag="xt",
                                            name=f"xt_0_{b}")
                            nc.scalar.copy(out=xt_b[:], in_=tp[:])
                            xt_tiles[(0, b)] = xt_b
                        xt_b = xt_tiles[(l, b)]

                        outp = opsp.tile([P, P], dt.float32, tag="outp",
                                         name=f"outp_{l}_{b}")
                        nc.tensor.matmul(out=outp[:], lhsT=wsl(l, 0), rhs=xt_b[:],
                                         start=True, stop=False)
                        nc.tensor.matmul(out=outp[:], lhsT=wsl(l, 1), rhs=gs[0][:],
                                         start=False, stop=False)
                        nc.tensor.matmul(out=outp[:], lhsT=wsl(l, 2), rhs=gs[1][:],
                                         start=False, stop=True)

                        if l < 2:
                            xt_nb = xtp.tile([P, P], dt.bfloat16, tag="xt",
                                             name=f"xt_{l + 1}_{b}")
                            nc.vector.tensor_scalar(
                                out=xt_nb[:], in0=outp[:],
                                scalar1=bsum_sb[:, l:l + 1], scalar2=None,
                                op0=mybir.AluOpType.add)
                            xt_tiles[(l + 1, b)] = xt_nb
                            tp2 = tpsp.tile([P, P], dt.bfloat16, tag="tp",
                                            name=f"tp_{l}_{b}")
                            nc.tensor.transpose(out=tp2[:], in_=xt_nb[:],
                                                identity=ident_sb[:])
                            rm = stagep.tile([P, P], dt.bfloat16, tag="rm",
                                             name=f"rm_{l}_{b}")
                            nc.scalar.copy(out=rm[:], in_=tp2[:])
                            nc.sync.dma_start(
                                out=agin[l][b * P:(b + 1) * P, :], in_=rm[:])
                        else:
                            o32 = stagep.tile([P, P], dt.float32, tag="o32",
                                              name=f"o32_{b}")
                            nc.vector.tensor_scalar(
                                out=o32[:], in0=outp[:],
                                scalar1=bsum_sb[:, 2:3], scalar2=None,
                                op0=mybir.AluOpType.add)
                            nc.sync.dma_start(
                                out=out_t.ap()[:, b * P:(b + 1) * P], in_=o32[:])

                if l < 2 and use_collective:
                    nc.gpsimd.collective_compute(
                        "AllGather",
                        mybir.AluOpType.bypass,
                        replica_groups=[list(range(NC))],
                        ins=[agin[l][:].opt()],
                        outs=[xfull[l][:].opt()],
                    )

    nc.compile()
    return nc


def _host_prep(x, edge_attr, edge_attr2, lnW, lnb, c1W, c1b, c2W, c2b,
               edge_index, edge_index2):
    x = np.asarray(x, np.float32)
    T1, off1, pc1 = _prep_edge_set(edge_index[0], edge_index[1], edge_attr)
    T2, off2, pc2 = _prep_edge_set(edge_index2[0], edge_index2[1], edge_attr2)
    plan = _Plan(T1, off1, T2, off2)

    xb = np.zeros((NFULL, F), BF16)
    xv = x.astype(BF16)
    for c in range(NC):
        xb[c * SHP:c * SHP + SH] = xv[c * SH:(c + 1) * SH]

    wall = np.zeros((P, 9 * F), BF16)
    for l in range(L):
        for k, W in enumerate((lnW, c1W, c2W)):
            wall[:, (l * 3 + k) * F:(l * 3 + k + 1) * F] = \
                np.asarray(W[l], np.float32).astype(BF16)
    bsum = np.stack([
        np.asarray(lnb[l], np.float32) + np.asarray(c1b[l], np.float32)
        + np.asarray(c2b[l], np.float32) for l in range(L)], axis=1)
    iota = np.tile(np.arange(P, dtype=BF16), (P, 1))
    ident = np.eye(P, dtype=BF16)

    in_maps = []
    for c in range(NC):
        m = {
            "xb": xb,
            "xown": xb[c * SHP:(c + 1) * SHP].copy(),
            "wall": wall,
            "bsum": np.ascontiguousarray(bsum, np.float32),
            "iota": iota,
            "ident": ident,
        }
        for s, pc in ((0, pc1), (1, pc2)):
            for h in (0, 1):
                m[f"idx{s}{h}"] = _wrap_idx(pc[c]["idx"][h])
            ncol = plan.ncols[s]
            dstc = np.zeros((P, ncol), BF16)
            wc = np.zeros((P, ncol), BF16)
            for h in (0, 1):
                nt = plan.tot[s][h]
                dstc[:, plan.hbase[s][h]:plan.hbase[s][h] + nt] = \
                    pc[c]["dstf"][h].reshape(nt, P).T.astype(BF16)
                wc[:, plan.hbase[s][h]:plan.hbase[s][h] + nt] = \
                    pc[c]["wf"][h].reshape(nt, P).T.astype(BF16)
            m[f"dst{s}"] = dstc
            m[f"w{s}"] = wc
        in_maps.append(m)
    return plan, in_maps


_CACHE = {}


def _get_compiled(plan_key, plan):
    if plan_key not in _CACHE:
        _CACHE[plan_key] = _build_nc(plan)
    return _CACHE[plan_key]


def kernel(x, edge_attr, edge_attr2, lnW, lnb, c1W, c1b, c2W, c2b,
           edge_index, edge_index2, batch):
    plan, in_maps = _host_prep(x, edge_attr, edge_attr2, lnW, lnb, c1W, c1b,
                               c2W, c2b, edge_index, edge_index2)
    key = (tuple(plan.T[0].ravel()), tuple(plan.T[1].ravel()))
    nc = _get_compiled(key, plan)
    res = bass_utils.run_bass_kernel_spmd(nc, in_maps, core_ids=list(range(NC)))
    out = np.empty((N, F), np.float32)
    for c in range(NC):
        out[c * SH:(c + 1) * SH] = res.results[c]["outT"].T[:SH]
    return out

